# revision 1
# baseline (speedup 1.0000x reference)
"""Trainium2 Bass kernel for differential flex self-attention (8-core TP over heads).

Contract: kernel(**inputs) takes the FULL unsharded inputs (as produced by the
problem's setup_inputs()) and returns the FULL [1, 2048, 2048] fp32 output.

Sharding (tensor parallel over heads, 8 NeuronCores):
  - core i owns v-heads {2i, 2i+1} == q/k dual-head pairs, i.e. rows
    [256*i, 256*(i+1)) of Wq/Wk/Wv.
  - Per core: q/k projections in transposed layout [feat, seq] and v in
    natural [seq, feat], RMS-norm + RoPE on q/k (dual 64-dim streams, q&k
    fused via strided APs), per-head dual-stream causal attention with scores
    computed transposed [k, q] (no max-subtraction needed: RMS-normalised q,k
    bound |score*scale| <= 8), exp on ACT, multiplicative causal mask on
    GpSimd, A^T = V^T P~^T on PE plus ones-matmul row-sums, scale-invariant
    differential combine rms(A1*s2 - lam*s1*A2), AllGather of A^T shards,
    out-projection against a 256-column shard of Wo.
  - Host: RoPE tables / mask tiles / scalar lambda, transposes of x and the
    weight shards, concat + transpose of the 8 output shards.
"""

import math

import numpy as np

N_CORES = 8
S = 2048          # sequence length
HID = 2048        # hidden size
QD = 64           # dual-head dim
HD = 128          # v head dim
FL = 256          # local q/k/v features per core (2 heads x 128)
NH_LOC = 2        # heads per core
LAMBDA_INIT = 0.8 - 0.6 * math.exp(-0.3 * 12)
SCALE = 1.0 / math.sqrt(QD)
EPS = float(np.finfo(np.float32).eps)
SC = 512          # seq chunk (matmul free dim)
NSC = S // SC     # 4
KT = 128          # key tile (partition dim)
NKT = S // KT     # 16
NKC = HID // 128  # contraction chunks for projections

# float32r (1 cycle/row on the PE when free dim >= 256) vs exact fp32
# (4 cycles/row). Flip to False if accuracy ever demands exact fp32 matmuls.
USE_F32R = True

_PROG_CACHE = {}


def _build_program():
    import concourse.mybir as mybir
    import concourse.tile as tile
    from concourse import bacc

    F32 = mybir.dt.float32
    R = mybir.dt.float32r
    EXP = mybir.ActivationFunctionType.Exp
    SQRT = mybir.ActivationFunctionType.Sqrt
    SQUARE = mybir.ActivationFunctionType.Square

    RD = R if USE_F32R else F32

    def _rsrc(ap):
        # bitcast a DMA source so both sides carry the matmul input dtype
        return ap.bitcast(RD) if USE_F32R else ap

    nc = bacc.Bacc("TRN2", target_bir_lowering=False, debug=False,
                   num_devices=N_CORES)

    # -------- I/O (per core) --------
    xT = nc.dram_tensor("xT", [HID, S], F32, kind="ExternalInput")
    WqT = nc.dram_tensor("WqT", [HID, FL], F32, kind="ExternalInput")
    WkT = nc.dram_tensor("WkT", [HID, FL], F32, kind="ExternalInput")
    WvT = nc.dram_tensor("WvT", [HID, FL], F32, kind="ExternalInput")
    WoT = nc.dram_tensor("WoT", [HID, FL], F32, kind="ExternalInput")
    cosT = nc.dram_tensor("cosT", [128, S], F32, kind="ExternalInput")
    sinT = nc.dram_tensor("sinT", [128, S], F32, kind="ExternalInput")
    m01 = nc.dram_tensor("m01", [KT, 4 * SC], F32, kind="ExternalInput")
    cgm_in = nc.dram_tensor("cgm", [128, 3], F32, kind="ExternalInput")
    gsel_in = nc.dram_tensor("gsel", [2, 128], F32, kind="ExternalInput")
    lam_in = nc.dram_tensor("lam", [1, 1], F32, kind="ExternalInput")
    outT = nc.dram_tensor("outT", [FL, S], F32, kind="ExternalOutput")
    # collective buffers (internal DRAM; output must be Shared)
    at_local = nc.dram_tensor("at_local", [FL, S], F32)
    at_full = nc.dram_tensor("at_full", [HID, S], F32, addr_space="Shared")

    with tile.TileContext(nc) as tc:
        with tc.tile_pool(name="const", bufs=1) as const:
            cgm = const.tile([128, 3], RD, tag="cgm", name="cgm")
            nc.sync.dma_start(cgm[:], _rsrc(cgm_in.ap())[:, :])
            ones = cgm[:, 0:1]
            gmask = cgm[:, 1:3]
            gsel = const.tile([2, 128], RD, tag="gsel", name="gsel")
            nc.sync.dma_start(gsel[:], _rsrc(gsel_in.ap())[:, :])
            eps_t = const.tile([128, 1], F32, tag="eps", name="eps")
            nc.any.memset(eps_t[:], EPS)

            cos_sb = const.tile([128, S], F32, tag="cos", name="cos")
            nc.sync.dma_start(cos_sb[:], cosT[:, :])
            sin_sb = const.tile([128, S], F32, tag="sin", name="sin")
            nc.sync.dma_start(sin_sb[:], sinT[:, :])
            m01_sb = const.tile([KT, 4 * SC], RD, tag="m01", name="m01")
            nc.sync.dma_start(m01_sb[:], _rsrc(m01.ap())[:, :])
            lam_sb = const.tile([1, 1], F32, tag="lam", name="lam")
            nc.sync.dma_start(lam_sb[:], lam_in[:, :])

            with tc.tile_pool(name="acts", bufs=1) as acts:
                # fused q|k transposed activations: cols [0,S) = qT,
                # [S,2S) = kT; row = local feature (head*... see slicing)
                qk = [acts.tile([128, 2 * S], RD, tag=f"qk{i}", name=f"qk{i}")
                      for i in range(2)]
                v_sb = acts.tile([128, NKT * FL], RD, tag="v", name="v")

                # ---------- Phase 1: projections + rms + rope ----------
                with tc.tile_pool(name="wpool", bufs=1) as wpool, \
                     tc.tile_pool(name="xpool", bufs=17) as xpool, \
                     tc.tile_pool(name="pj_ps", bufs=3, space="PSUM") as pj_ps, \
                     tc.tile_pool(name="v_ps", bufs=2, space="PSUM") as v_ps, \
                     tc.tile_pool(name="g_ps", bufs=2, space="PSUM") as g_ps, \
                     tc.tile_pool(name="ev", bufs=3) as ev, \
                     tc.tile_pool(name="evs", bufs=2) as evs:

                    def load_w(wname, dram):
                        t = wpool.tile([128, NKC * FL], RD, tag=wname,
                                       name=wname)
                        nc.sync.dma_start(
                            t[:],
                            _rsrc(dram.ap()).rearrange("(kc p) f -> p kc f",
                                                       p=128))
                        return t

                    wq_sb = load_w("wq", WqT)
                    wk_sb = load_w("wk", WkT)
                    wv_sb = load_w("wv", WvT)

                    for sc in range(NSC):
                        xts = []
                        for kc in range(NKC):
                            xt = xpool.tile([128, SC], RD, tag="xt", name="xt")
                            nc.sync.dma_start(
                                xt[:],
                                _rsrc(xT.ap())[kc * 128:(kc + 1) * 128,
                                               sc * SC:(sc + 1) * SC])
                            xts.append(xt)

                        # ---- v in natural [seq, feat] layout:
                        # stationary xT tile, moving Wv chunk
                        for j in range(SC // 128):
                            stile = sc * (SC // 128) + j
                            vp = v_ps.tile([128, FL], F32, tag="vps",
                                           name="vps")
                            for kc in range(NKC):
                                nc.tensor.matmul(
                                    vp[:],
                                    xts[kc][:, j * 128:(j + 1) * 128],
                                    wv_sb[:, kc * FL:(kc + 1) * FL],
                                    start=(kc == 0), stop=(kc == NKC - 1))
                            nc.scalar.copy(
                                v_sb[:, stile * FL:(stile + 1) * FL], vp[:])

                        # ---- q and k (transposed layout, paired per ft)
                        for ft in range(2):
                            psq = pj_ps.tile([128, SC], F32, tag="pjps",
                                             name="psq")
                            psk = pj_ps.tile([128, SC], F32, tag="pjps",
                                             name="psk")
                            for kc in range(NKC):
                                nc.tensor.matmul(
                                    psq[:],
                                    wq_sb[:, kc * FL + ft * 128:
                                          kc * FL + (ft + 1) * 128],
                                    xts[kc][:],
                                    start=(kc == 0), stop=(kc == NKC - 1))
                            for kc in range(NKC):
                                nc.tensor.matmul(
                                    psk[:],
                                    wk_sb[:, kc * FL + ft * 128:
                                          kc * FL + (ft + 1) * 128],
                                    xts[kc][:],
                                    start=(kc == 0), stop=(kc == NKC - 1))

                            # rms factors for q and k -> fused qn [128, 2*SC]
                            qn = evs.tile([128, 2 * SC], F32, tag="qn",
                                          name="qn")
                            for which, pst in ((0, psq), (1, psk)):
                                sq = evs.tile([128, SC], RD, tag="sq",
                                              name="sq")
                                nc.scalar.activation(sq[:], pst[:], SQUARE)
                                gs = g_ps.tile([2, SC], F32, tag="gs",
                                               name="gs")
                                nc.tensor.matmul(gs[:], gmask, sq[:],
                                                 start=True, stop=True)
                                fac = evs.tile([2, SC], F32, tag="fac",
                                               name="fac")
                                nc.scalar.activation(
                                    fac[:], gs[:], SQRT,
                                    scale=1.0 / QD, bias=eps_t[0:2, :])
                                rc2 = evs.tile([2, SC], RD, tag="rc2",
                                               name="rc2")
                                with nc.allow_low_precision(
                                        reason="f32r rounding for matmul rhs"):
                                    nc.vector.reciprocal(rc2[:], fac[:])
                                fb = g_ps.tile([128, SC], F32, tag="fb",
                                               name="fb", bufs=1)
                                nc.tensor.matmul(fb[:], gsel[:], rc2[:],
                                                 start=True, stop=True)
                                fbs = evs.tile([128, SC], F32, tag="fbs",
                                               name="fbs")
                                nc.scalar.copy(fbs[:], fb[:])
                                nc.vector.tensor_mul(
                                    qn[:, which * SC:(which + 1) * SC],
                                    pst[:], fbs[:])

                            # fused rope over q|k halves (strided free APs)
                            dst = qk[ft]
                            # destination free pattern: two 512-col chunks at
                            # stride S (q chunk at sc*SC, k chunk at S+sc*SC)
                            def dslice(p0, p1):
                                return dst[p0:p1, :].rearrange(
                                    "p (t s) -> p t s", t=2)[
                                    :, :, sc * SC:(sc + 1) * SC]
                            qn3 = qn.rearrange("p (t s) -> p t s", t=2)
                            cs3 = cos_sb[:, sc * SC:(sc + 1) * SC]
                            sn3 = sin_sb[:, sc * SC:(sc + 1) * SC]
                            for st in range(2):
                                b = st * QD
                                x1 = qn3[b:b + 32, :, :]
                                x2 = qn3[b + 32:b + 64, :, :]
                                c_lo = cs3[b:b + 32, :].unsqueeze(1) \
                                    .to_broadcast([32, 2, SC])
                                s_lo = sn3[b:b + 32, :].unsqueeze(1) \
                                    .to_broadcast([32, 2, SC])
                                c_hi = cs3[b + 32:b + 64, :].unsqueeze(1) \
                                    .to_broadcast([32, 2, SC])
                                s_hi = sn3[b + 32:b + 64, :].unsqueeze(1) \
                                    .to_broadcast([32, 2, SC])
                                rt1 = evs.tile([128, 2 * SC], F32, tag="rt1",
                                               name="rt1", bufs=1)
                                rt2 = evs.tile([128, 2 * SC], F32, tag="rt2",
                                               name="rt2", bufs=1)
                                t1 = rt1.rearrange("p (t s) -> p t s", t=2)
                                t2 = rt2.rearrange("p (t s) -> p t s", t=2)
                                # y1 = x1*cos + x2*sin   (write rows b..b+32)
                                nc.vector.tensor_mul(t1[b:b + 32], x1, c_lo)
                                nc.vector.tensor_mul(t2[b:b + 32], x2, s_hi)
                                nc.vector.tensor_add(
                                    dslice(b, b + 32),
                                    t1[b:b + 32], t2[b:b + 32])
                                # y2 = x2*cos - x1*sin  (write rows b+32..b+64)
                                nc.vector.tensor_mul(
                                    t1[b + 32:b + 64], x2, c_hi)
                                nc.vector.tensor_mul(
                                    t2[b + 32:b + 64], x1, s_lo)
                                nc.vector.tensor_sub(
                                    dslice(b + 32, b + 64),
                                    t1[b + 32:b + 64], t2[b + 32:b + 64])

                # ---------- Phase 2: attention ----------
                with tc.tile_pool(name="sc_ps", bufs=3, space="PSUM") as sc_ps, \
                     tc.tile_pool(name="at_ps", bufs=3, space="PSUM") as at_ps, \
                     tc.tile_pool(name="sm_ps", bufs=2, space="PSUM") as sm_ps, \
                     tc.tile_pool(name="pexp", bufs=6) as pexp, \
                     tc.tile_pool(name="cb", bufs=2) as cb:

                    for h in range(NH_LOC):
                        qTh = qk[h][:, 0:S]
                        kTh = qk[h][:, S:2 * S]
                        for qc in range(NSC):
                            nkt = (qc + 1) * (SC // 128)
                            atp = [None, None]
                            ssb = [None, None]
                            for st in range(2):
                                a = at_ps.tile([128, SC], F32, tag="atps",
                                               name="atps")
                                smp = sm_ps.tile([1, SC], F32, tag="smps",
                                                 name="smps")
                                for kt in range(nkt):
                                    scp = sc_ps.tile([128, SC], F32,
                                                     tag="scps", name="scps")
                                    nc.tensor.matmul(
                                        scp[:],
                                        kTh[st * QD:(st + 1) * QD,
                                            kt * 128:(kt + 1) * 128],
                                        qTh[st * QD:(st + 1) * QD,
                                            qc * SC:(qc + 1) * SC],
                                        start=True, stop=True)
                                    pe = pexp.tile([128, SC], RD, tag="pexp",
                                                   name="pexp")
                                    nc.scalar.activation(pe[:], scp[:], EXP,
                                                         scale=SCALE)
                                    off_idx = kt - qc * (SC // 128)
                                    if off_idx >= 0:
                                        pem = pexp.tile([128, SC], RD,
                                                        tag="pem", name="pem")
                                        nc.gpsimd.tensor_mul(
                                            pem[:], pe[:],
                                            m01_sb[:, off_idx * SC:
                                                   (off_idx + 1) * SC])
                                        pe = pem
                                    nc.tensor.matmul(
                                        a[:],
                                        v_sb[:, kt * FL + h * 128:
                                             kt * FL + (h + 1) * 128],
                                        pe[:],
                                        start=(kt == 0), stop=(kt == nkt - 1))
                                    nc.tensor.matmul(
                                        smp[:], ones, pe[:],
                                        start=(kt == 0), stop=(kt == nkt - 1))
                                s_sb = cb.tile([1, SC], F32, tag=f"s{st}",
                                               name=f"s{st}")
                                nc.scalar.copy(s_sb[:], smp[:])
                                atp[st] = a
                                ssb[st] = s_sb
                            # scale-invariant combine:
                            # comb = A1*s2 - (lam*s1)*A2  (rms-equivalent)
                            w1 = cb.tile([1, SC], F32, tag="w1", name="w1")
                            nc.vector.tensor_scalar_mul(w1[:], ssb[0][:],
                                                        lam_sb[:])
                            ub0 = cb.tile([128, SC], F32, tag="ub0",
                                          name="ub0")
                            nc.gpsimd.partition_broadcast(ub0[:],
                                                          ssb[1][0:1, :])
                            ub1 = cb.tile([128, SC], F32, tag="ub1",
                                          name="ub1")
                            nc.gpsimd.partition_broadcast(ub1[:], w1[0:1, :])
                            ta = cb.tile([128, SC], F32, tag="ta", name="ta")
                            nc.vector.tensor_mul(ta[:], atp[0][:], ub0[:])
                            tb = cb.tile([128, SC], F32, tag="tb", name="tb")
                            nc.vector.tensor_mul(tb[:], atp[1][:], ub1[:])
                            comb = cb.tile([128, SC], F32, tag="comb",
                                           name="comb")
                            nc.vector.tensor_sub(comb[:], ta[:], tb[:])
                            sqc = cb.tile([128, SC], RD, tag="sqc",
                                          name="sqc")
                            nc.scalar.activation(sqc[:], comb[:], SQUARE)
                            gps = sm_ps.tile([1, SC], F32, tag="smps",
                                             name="gps")
                            nc.tensor.matmul(gps[:], ones, sqc[:],
                                             start=True, stop=True)
                            rf = cb.tile([1, SC], F32, tag="rf", name="rf")
                            nc.scalar.activation(rf[:], gps[:], SQRT,
                                                 scale=1.0 / HD,
                                                 bias=eps_t[0:1, :])
                            rf2 = cb.tile([1, SC], F32, tag="rf2", name="rf2")
                            nc.vector.reciprocal(rf2[:], rf[:])
                            nc.scalar.mul(rf2[:], rf2[:], 1.0 - LAMBDA_INIT)
                            rb = cb.tile([128, SC], F32, tag="rb", name="rb")
                            nc.gpsimd.partition_broadcast(rb[:], rf2[0:1, :])
                            ot = cb.tile([128, SC], F32, tag="ot", name="ot")
                            nc.vector.tensor_mul(ot[:], comb[:], rb[:])
                            nc.sync.dma_start(
                                at_local[h * 128:(h + 1) * 128,
                                         qc * SC:(qc + 1) * SC], ot[:])

            # ---------- Phase 3: AllGather + out-projection ----------
            nc.gpsimd.collective_compute(
                "AllGather", mybir.AluOpType.bypass,
                replica_groups=[list(range(N_CORES))],
                ins=[at_local.ap().opt()], outs=[at_full.ap().opt()],
            )

            with tc.tile_pool(name="afpool", bufs=18) as afpool, \
                 tc.tile_pool(name="op_ps", bufs=2, space="PSUM") as op_ps, \
                 tc.tile_pool(name="oevp", bufs=3) as oevp:
                wo_sb = afpool.tile([128, NKC * FL], RD, tag="wo", name="wo",
                                    bufs=1)
                nc.sync.dma_start(
                    wo_sb[:],
                    _rsrc(WoT.ap()).rearrange("(kc p) f -> p kc f", p=128))
                for sc2 in range(NSC):
                    afs = []
                    for kc in range(NKC):
                        af = afpool.tile([128, SC], RD, tag="af", name="af")
                        nc.sync.dma_start(
                            af[:],
                            _rsrc(at_full.ap())[kc * 128:(kc + 1) * 128,
                                                sc2 * SC:(sc2 + 1) * SC])
                        afs.append(af)
                    for oft in range(2):
                        ps = op_ps.tile([128, SC], F32, tag="opps",
                                        name="opps")
                        for kc in range(NKC):
                            nc.tensor.matmul(
                                ps[:],
                                wo_sb[:, kc * FL + oft * 128:
                                      kc * FL + (oft + 1) * 128],
                                afs[kc][:],
                                start=(kc == 0), stop=(kc == NKC - 1))
                        oev = oevp.tile([128, SC], F32, tag="oev", name="oev")
                        nc.scalar.copy(oev[:], ps[:])
                        nc.sync.dma_start(
                            outT[oft * 128:(oft + 1) * 128,
                                 sc2 * SC:(sc2 + 1) * SC],
                            oev[:])

    nc.compile()
    return nc


def _get_program():
    if "nc" not in _PROG_CACHE:
        _PROG_CACHE["nc"] = _build_program()
    return _PROG_CACHE["nc"]


def _host_inputs(x, x_pos, Wq, Wk, Wv, Wo, lq1, lk1, lq2, lk2):
    x = np.asarray(x, dtype=np.float32)
    xT = np.ascontiguousarray(x.reshape(S, HID).T)

    pos = np.asarray(x_pos, dtype=np.float32).reshape(S)
    inv_freq = (1.0 / (10000.0 ** (np.arange(0, QD, 2, dtype=np.float32) / QD))
                ).astype(np.float32)
    freqs = pos[:, None] * inv_freq[None, :]          # [S, 32]
    cos32 = np.cos(freqs).astype(np.float32).T        # [32, S]
    sin32 = np.sin(freqs).astype(np.float32).T
    cosT = np.ascontiguousarray(np.tile(cos32, (4, 1)))   # [128, S]
    sinT = np.ascontiguousarray(np.tile(sin32, (4, 1)))

    lq1 = np.asarray(lq1, np.float32); lk1 = np.asarray(lk1, np.float32)
    lq2 = np.asarray(lq2, np.float32); lk2 = np.asarray(lk2, np.float32)
    lam = (np.exp(np.sum(lq1 * lk1, dtype=np.float32), dtype=np.float32)
           - np.exp(np.sum(lq2 * lk2, dtype=np.float32), dtype=np.float32)
           + np.float32(LAMBDA_INIT))
    lam = np.array([[lam]], dtype=np.float32)

    cgm = np.zeros((128, 3), dtype=np.float32)
    cgm[:, 0] = 1.0        # ones column (row-sum matmuls)
    cgm[0:64, 1] = 1.0     # rms group mask: stream 0
    cgm[64:128, 2] = 1.0   # rms group mask: stream 1
    gsel = np.zeros((2, 128), dtype=np.float32)
    gsel[0, 0:64] = 1.0
    gsel[1, 64:128] = 1.0

    kk = np.arange(KT, dtype=np.int64)[:, None]
    qq = np.arange(SC, dtype=np.int64)[None, :]
    m01 = np.concatenate(
        [(qq - kk >= off * KT).astype(np.float32)
         for off in range(4)], axis=1)                # [128, 4*512]

    Wq = np.asarray(Wq, np.float32); Wk = np.asarray(Wk, np.float32)
    Wv = np.asarray(Wv, np.float32); Wo = np.asarray(Wo, np.float32)

    in_maps = []
    for i in range(N_CORES):
        sl = slice(i * FL, (i + 1) * FL)
        in_maps.append({
            "xT": xT,
            "WqT": np.ascontiguousarray(Wq[sl, :].T),
            "WkT": np.ascontiguousarray(Wk[sl, :].T),
            "WvT": np.ascontiguousarray(Wv[sl, :].T),
            "WoT": np.ascontiguousarray(Wo[sl, :].T),
            "cosT": cosT, "sinT": sinT, "m01": m01, "cgm": cgm,
            "gsel": gsel, "lam": lam,
        })
    return in_maps


def kernel(x, x_pos, Wq, Wk, Wv, Wo, lq1, lk1, lq2, lk2):
    from concourse.bass_utils import run_bass_kernel_spmd

    nc = _get_program()
    in_maps = _host_inputs(x, x_pos, Wq, Wk, Wv, Wo, lq1, lk1, lq2, lk2)
    res = run_bass_kernel_spmd(nc, in_maps, list(range(N_CORES)))
    outT_full = np.concatenate(
        [res.results[c]["outT"] for c in range(N_CORES)], axis=0)  # [HID, S]
    return np.ascontiguousarray(outT_full.T).reshape(1, S, HID)



# revision 2
# speedup vs baseline: 10148.9956x; 10148.9956x over previous
"""Trainium2 Bass kernel v3 for differential flex self-attention (8-core TP).

kernel(**inputs) takes FULL unsharded inputs, returns FULL [1,2048,2048] f32.

v3 = v2 + fused phase-1/phase-2 emission per seq chunk: attention for query
chunk qc runs right after projection chunk sc=qc (causality makes all its
keys available), so the chunked AllGather chain starts ~50us into the kernel
and hides under later compute.  Pool/GpSimd queue carries ONLY collectives
(+ a few pre-phase constant DMAs); RoPE moves entirely to DVE.  Optional fp8
payload for the AG + out-projection (DoubleRow matmul).
"""

import math

import numpy as np

N_CORES = 8
S = 2048
HID = 2048
QD = 64
HD = 128
FL = 256
NH_LOC = 2
LAMBDA_INIT = 0.8 - 0.6 * math.exp(-0.3 * 12)
SCALE = 1.0 / math.sqrt(QD)
EPS = float(np.finfo(np.float32).eps)
SC = 512
NSC = S // SC
KT = 128
NKC = HID // 128

import os
FP8_AT = os.environ.get("K3_FP8", "0") == "1"   # fp8 AG payload (hurts accuracy)
SKIP_AG = os.environ.get("K3_SKIP_AG", "0") == "1"  # timing probe only
# One AllGather at the end: on TRN2 each collective stalls compute while
# active (~95us per mid-kernel chunked AG vs ~31us for a single tail AG).
AG_GROUP = int(os.environ.get("K3_AG_GROUP", "4"))

_PROG_CACHE = {}

_SWAP16 = [(i + 16) % 32 for i in range(32)]


def _build_program(reps: int = 1):
    import concourse.mybir as mybir
    import concourse.tile as tile
    from concourse import bacc

    F32 = mybir.dt.float32
    R = mybir.dt.float32r
    BF = mybir.dt.bfloat16
    AT = mybir.dt.float8e4 if FP8_AT else BF

    nc = bacc.Bacc("TRN2", target_bir_lowering=False, debug=False,
                   num_devices=N_CORES)

    xT = nc.dram_tensor("xT", [HID, S], F32, kind="ExternalInput")
    WqT = nc.dram_tensor("WqT", [HID, FL], F32, kind="ExternalInput")
    WkT = nc.dram_tensor("WkT", [HID, FL], F32, kind="ExternalInput")
    WvT = nc.dram_tensor("WvT", [HID, FL], F32, kind="ExternalInput")
    WoT = nc.dram_tensor("WoT", [HID, FL], AT, kind="ExternalInput")
    cosT = nc.dram_tensor("cosT", [128, S], F32, kind="ExternalInput")
    sinT = nc.dram_tensor("sinT", [128, S], F32, kind="ExternalInput")
    m01 = nc.dram_tensor("m01", [KT, 4 * SC], BF, kind="ExternalInput")
    cst_in = nc.dram_tensor("cst", [128, 4], F32, kind="ExternalInput")
    onesb_in = nc.dram_tensor("onesb", [128, 1], BF, kind="ExternalInput")
    gsel_in = nc.dram_tensor("gsel", [2, 128], F32, kind="ExternalInput")
    ones1_in = nc.dram_tensor("ones1", [1, 128], F32, kind="ExternalInput")
    lam_in = nc.dram_tensor("lam", [1, 1], F32, kind="ExternalInput")
    outT = nc.dram_tensor("outT", [FL, S], F32, kind="ExternalOutput")
    ng = NSC // AG_GROUP
    at_loc = [nc.dram_tensor(f"at_loc{i}", [FL, AG_GROUP * SC], AT)
              for i in range(ng)]
    at_full = [nc.dram_tensor(f"at_full{i}", [HID, AG_GROUP * SC], AT,
                              addr_space="Shared") for i in range(ng)]

    def rsrc(ap):
        return ap.bitcast(R)

    with tile.TileContext(nc) as tc:
        with tc.tile_pool(name="const", bufs=1) as const:
            cst = const.tile([128, 4], R, tag="cst", name="cst")
            nc.sync.dma_start(cst[:], rsrc(cst_in.ap())[:, :])
            onesc = cst[:, 0:1]
            gmask = cst[:, 1:3]
            onesb = const.tile([128, 1], BF, tag="onesb", name="onesb")
            nc.sync.dma_start(onesb[:], onesb_in[:, :])
            gsel = const.tile([2, 128], R, tag="gsel", name="gsel")
            nc.sync.dma_start(gsel[:], rsrc(gsel_in.ap())[:, :])
            ones1 = const.tile([1, 128], R, tag="ones1", name="ones1")
            nc.sync.dma_start(ones1[:], rsrc(ones1_in.ap())[:, :])
            lam_sb = const.tile([1, 1], F32, tag="lam", name="lam")
            nc.sync.dma_start(lam_sb[:], lam_in[:, :])
            eps_t = const.tile([128, 1], F32, tag="eps", name="eps")
            nc.any.memset(eps_t[:], EPS)
            s2c = 1.0 / (1.0 - LAMBDA_INIT) ** 2
            epss_t = const.tile([128, 1], F32, tag="epss", name="epss")
            nc.any.memset(epss_t[:], EPS * s2c)
            ones_t = const.tile([128, 1], F32, tag="onest", name="onest")
            nc.any.memset(ones_t[:], 1.0)
            # big constants on the gpsimd queue: it is otherwise idle until
            # the first AllGather (~50us in), and these all land well before.
            cos_sb = const.tile([128, S], F32, tag="cos", name="cos")
            nc.gpsimd.dma_start(cos_sb[:], cosT[:, :])
            sin_sb = const.tile([128, S], F32, tag="sin", name="sin")
            nc.gpsimd.dma_start(sin_sb[:], sinT[:, :])
            m01_sb = const.tile([KT, 4 * SC], BF, tag="m01", name="m01")
            nc.gpsimd.dma_start(m01_sb[:], m01.ap()[:, :])

            with tc.tile_pool(name="wts", bufs=1) as wts:
                def load_w(wname, dram, dt, engine):
                    t = wts.tile([128, NKC * FL], dt, tag=wname, name=wname)
                    src = dram.ap() if dt != R else rsrc(dram.ap())
                    # split into 4 chunk DMAs so the first matmuls can start
                    # after ~1/4 of the load
                    for c in range(4):
                        engine.dma_start(
                            t[:, c * 4 * FL:(c + 1) * 4 * FL],
                            src.rearrange("(kc p) f -> p kc f", p=128)
                            [:, c * 4:(c + 1) * 4, :])
                    return t

                wq_sb = load_w("wq", WqT, R, nc.sync)
                wk_sb = load_w("wk", WkT, R, nc.sync)
                wv_sb = load_w("wv", WvT, R, nc.sync)
                wo_sb = load_w("wo", WoT, AT, nc.gpsimd)

                for _rep in range(reps):
                    _emit_body(nc, tc, mybir, tile, locals())

    nc.compile()
    return nc


def _emit_body(nc, tc, mybir, tile, env):
    F32 = mybir.dt.float32
    R = mybir.dt.float32r
    BF = mybir.dt.bfloat16
    AT = mybir.dt.float8e4 if FP8_AT else BF
    EXP = mybir.ActivationFunctionType.Exp
    SQRT = mybir.ActivationFunctionType.Sqrt
    SQUARE = mybir.ActivationFunctionType.Square
    MUL = mybir.AluOpType.mult
    BYP = mybir.AluOpType.bypass
    DR = None  # DoubleRow needs packed operand layouts; fp8 still halves AG bytes

    xT = env["xT"]; cos_sb = env["cos_sb"]; sin_sb = env["sin_sb"]
    m01_sb = env["m01_sb"]; lam_sb = env["lam_sb"]
    onesc = env["onesc"]; gmask = env["gmask"]; onesb = env["onesb"]
    gsel = env["gsel"]; ones1 = env["ones1"]
    wq_sb = env["wq_sb"]; wk_sb = env["wk_sb"]; wv_sb = env["wv_sb"]
    wo_sb = env["wo_sb"]
    at_loc = env["at_loc"]; at_full = env["at_full"]; outT = env["outT"]
    eps_t = env["eps_t"]; epss_t = env["epss_t"]; ones_t = env["ones_t"]

    def rsrc(ap):
        return ap.bitcast(R)

    with tc.tile_pool(name="acts", bufs=1) as acts:
        qk = [acts.tile([128, 2 * S], BF, tag=f"qk{i}", name=f"qk{i}")
              for i in range(2)]
        v_sb = acts.tile([128, (S // KT) * FL], BF, tag="v", name="v")

        with tc.tile_pool(name="xpool", bufs=17) as xpool, \
             tc.tile_pool(name="pj_ps", bufs=2, space="PSUM") as pj_ps, \
             tc.tile_pool(name="wk_ps", bufs=2, space="PSUM") as wk_ps, \
             tc.tile_pool(name="at_ps", bufs=2, space="PSUM") as at_ps, \
             tc.tile_pool(name="sm_ps", bufs=1, space="PSUM") as sm_ps, \
             tc.tile_pool(name="bc_ps", bufs=1, space="PSUM") as bc_ps, \
             tc.tile_pool(name="ev", bufs=2) as ev, \
             tc.tile_pool(name="rp", bufs=2) as rp, \
             tc.tile_pool(name="pexp", bufs=4) as pexp, \
             tc.tile_pool(name="cb1", bufs=1) as cb1, \
             tc.tile_pool(name="cb", bufs=2) as cb:

            for sc in range(NSC):
                # ---------- phase 1 slice: proj + rms + rope for chunk sc
                xts = []
                for kc in range(NKC):
                    xt = xpool.tile([128, SC], R, tag="xt", name="xt")
                    nc.sync.dma_start(
                        xt[:], rsrc(xT.ap())[kc * 128:(kc + 1) * 128,
                                             sc * SC:(sc + 1) * SC])
                    xts.append(xt)

                for j in range(SC // 128):
                    stile = sc * (SC // 128) + j
                    vp = wk_ps.tile([128, SC], F32, tag="wk", name="vp")
                    for kc in range(NKC):
                        nc.tensor.matmul(
                            vp[:, 0:FL], xts[kc][:, j * 128:(j + 1) * 128],
                            wv_sb[:, kc * FL:(kc + 1) * FL],
                            start=(kc == 0), stop=(kc == NKC - 1))
                    with nc.allow_low_precision(reason="bf16 v"):
                        nc.scalar.copy(
                            v_sb[:, stile * FL:(stile + 1) * FL],
                            vp[:, 0:FL])

                for ft in range(2):
                    psq = pj_ps.tile([128, SC], F32, tag="pj", name="psq")
                    psk = pj_ps.tile([128, SC], F32, tag="pj", name="psk")
                    for kc in range(NKC):
                        nc.tensor.matmul(
                            psq[:],
                            wq_sb[:, kc * FL + ft * 128:
                                  kc * FL + (ft + 1) * 128],
                            xts[kc][:], start=(kc == 0), stop=(kc == NKC - 1))
                    for kc in range(NKC):
                        nc.tensor.matmul(
                            psk[:],
                            wk_sb[:, kc * FL + ft * 128:
                                  kc * FL + (ft + 1) * 128],
                            xts[kc][:], start=(kc == 0), stop=(kc == NKC - 1))

                    for which, pst in ((0, psq), (1, psk)):
                        sq = ev.tile([128, SC], R, tag="sq", name="sq")
                        nc.scalar.activation(sq[:], pst[:], SQUARE)
                        gs = wk_ps.tile([2, SC], F32, tag="wk", name="gs")
                        nc.tensor.matmul(gs[0:2, :], gmask, sq[:],
                                         start=True, stop=True)
                        fac = ev.tile([2, SC], F32, tag="fac", name="fac")
                        nc.scalar.activation(fac[:], gs[0:2, :], SQRT,
                                             scale=1.0 / QD,
                                             bias=eps_t[0:2, :])
                        rc2 = ev.tile([2, SC], R, tag="rc2", name="rc2")
                        with nc.allow_low_precision(reason="f32r rhs"):
                            nc.vector.reciprocal(rc2[:], fac[:])
                        fb = wk_ps.tile([128, SC], F32, tag="wk", name="fb")
                        nc.tensor.matmul(fb[:], gsel[:], rc2[:],
                                         start=True, stop=True)
                        fbs = ev.tile([128, SC], F32, tag="fbs", name="fbs")
                        nc.scalar.copy(fbs[:], fb[:])
                        qn = rp.tile([128, SC], F32, tag="qn", name="qn")
                        nc.vector.tensor_mul(qn[:], pst[:], fbs[:])
                        shn = rp.tile([128, SC], F32, tag="shn", name="shn")
                        nc.vector.stream_shuffle(shn[:], qn[:], _SWAP16)
                        cs = cos_sb[:, sc * SC:(sc + 1) * SC]
                        sn = sin_sb[:, sc * SC:(sc + 1) * SC]
                        t1 = rp.tile([128, SC], F32, tag="t1", name="t1")
                        nc.vector.tensor_mul(t1[:], qn[:], cs)
                        t2 = rp.tile([128, SC], F32, tag="t2", name="t2")
                        nc.vector.tensor_mul(t2[:], shn[:], sn)
                        dst = qk[ft][:, which * S + sc * SC:
                                     which * S + (sc + 1) * SC]
                        with nc.allow_low_precision(reason="bf16 qk"):
                            nc.vector.tensor_add(dst, t1[:], t2[:])

                # ---------- phase 2 slice: attention for qc = sc
                qc = sc
                nkt = (qc + 1) * (SC // 128)
                combs = []
                sms = []
                for h in range(NH_LOC):
                    qTh = qk[h][:, 0:S]
                    kTh = qk[h][:, S:2 * S]
                    atp = [None, None]
                    for st in range(2):
                        a = at_ps.tile([128, SC], F32, tag="atps",
                                       name="atps")
                        smp = sm_ps.tile([1, SC], F32, tag="smps",
                                         name="smps")
                        for kt in range(nkt):
                            scp = wk_ps.tile([128, SC], F32, tag="wk",
                                             name="scp")
                            nc.tensor.matmul(
                                scp[:],
                                kTh[st * QD:(st + 1) * QD,
                                    kt * 128:(kt + 1) * 128],
                                qTh[st * QD:(st + 1) * QD,
                                    qc * SC:(qc + 1) * SC],
                                start=True, stop=True)
                            pe = pexp.tile([128, SC], BF, tag="pexp",
                                           name="pexp")
                            with nc.allow_low_precision(reason="bf16 exp"):
                                nc.scalar.activation(pe[:], scp[:], EXP,
                                                     scale=SCALE)
                            off_idx = kt - qc * (SC // 128)
                            if off_idx >= 0:
                                pem = pexp.tile([128, SC], BF, tag="pem",
                                                name="pem")
                                with nc.allow_low_precision(
                                        reason="bf16 mask"):
                                    nc.vector.tensor_mul(
                                        pem[:], pe[:],
                                        m01_sb[:, off_idx * SC:
                                               (off_idx + 1) * SC])
                                pe = pem
                            nc.tensor.matmul(
                                a[:],
                                v_sb[:, kt * FL + h * 128:
                                     kt * FL + (h + 1) * 128],
                                pe[:], start=(kt == 0),
                                stop=(kt == nkt - 1))
                            nc.tensor.matmul(
                                smp[:], onesb, pe[:],
                                start=(kt == 0), stop=(kt == nkt - 1))
                        atp[st] = a
                        if st == 0:
                            # free smp(st0) early: fold lam in now
                            w1 = cb1.tile([1, SC], R, tag="w1", name="w1")
                            with nc.allow_low_precision(reason="f32r w1"):
                                nc.vector.tensor_scalar_mul(w1[:], smp[:],
                                                            lam_sb[:])
                            sms.append(("w1", w1))
                        else:
                            s2_sb = cb1.tile([1, SC], R, tag="s2sb",
                                            name="s2sb")
                            with nc.allow_low_precision(reason="f32r s"):
                                nc.vector.tensor_scalar_mul(
                                    s2_sb[:], smp[:], ones_t[0:1, :])
                            sms.append(("s2", s2_sb))
                    w1 = sms[-2][1]
                    s2_sb = sms[-1][1]
                    bc2 = bc_ps.tile([128, SC], F32, tag="bc", name="bc2")
                    nc.tensor.matmul(bc2[:], ones1, s2_sb[:],
                                     start=True, stop=True)
                    bc2s = cb1.tile([128, SC], F32, tag="bcs", name="bc2s")
                    nc.vector.tensor_scalar_mul(bc2s[:], bc2[:], ones_t[:])
                    ta = cb1.tile([128, SC], F32, tag="ta", name="ta")
                    nc.vector.tensor_mul(ta[:], atp[0][:], bc2s[:])
                    bc1 = bc_ps.tile([128, SC], F32, tag="bc", name="bc1")
                    nc.tensor.matmul(bc1[:], ones1, w1[:],
                                     start=True, stop=True)
                    bc1s = cb1.tile([128, SC], F32, tag="bcs", name="bc1s")
                    nc.vector.tensor_scalar_mul(bc1s[:], bc1[:], ones_t[:])
                    tb = cb1.tile([128, SC], F32, tag="tb", name="tb")
                    nc.vector.tensor_mul(tb[:], atp[1][:], bc1s[:])
                    comb = cb.tile([128, SC], F32, tag="comb",
                                   name=f"comb{h}")
                    nc.vector.tensor_sub(comb[:], ta[:], tb[:])
                    combs.append(comb)
                # rms sums + sqrt pair + final scale + at_loc store
                gpss = []
                for h in range(NH_LOC):
                    sqc = cb1.tile([128, SC], R, tag="sqc", name="sqc")
                    nc.scalar.activation(sqc[:], combs[h][:], SQUARE)
                    gps = sm_ps.tile([1, SC], F32, tag="smps", name="gps")
                    nc.tensor.matmul(gps[:], onesc, sqc[:],
                                     start=True, stop=True)
                    gpss.append(gps)
                s2 = 1.0 / (1.0 - LAMBDA_INIT) ** 2
                rfs = []
                for h in range(NH_LOC):
                    rf = cb.tile([1, SC], F32, tag="rf", name="rf")
                    nc.scalar.activation(rf[:], gpss[h][:], SQRT,
                                         scale=s2 / HD,
                                         bias=epss_t[0:1, :])
                    rfs.append(rf)
                for h in range(NH_LOC):
                    rf2 = cb.tile([1, SC], R, tag="rf2", name="rf2")
                    with nc.allow_low_precision(reason="f32r rf2"):
                        nc.vector.reciprocal(rf2[:], rfs[h][:])
                    rb = bc_ps.tile([128, SC], F32, tag="bc", name="rb")
                    nc.tensor.matmul(rb[:], ones1, rf2[:],
                                     start=True, stop=True)
                    rbs = cb1.tile([128, SC], F32, tag="bcs", name="rbs")
                    nc.vector.tensor_scalar_mul(rbs[:], rb[:], ones_t[:])
                    ot = cb.tile([128, SC], AT, tag="ot", name="ot")
                    with nc.allow_low_precision(reason="fp8/bf16 at"):
                        nc.vector.tensor_mul(ot[:], combs[h][:], rbs[:])
                    nc.sync.dma_start(
                        at_loc[qc // AG_GROUP].ap()
                        [h * 128:(h + 1) * 128,
                         (qc % AG_GROUP) * SC:(qc % AG_GROUP + 1) * SC],
                        ot[:])
                if not SKIP_AG and qc % AG_GROUP == AG_GROUP - 1:
                    g = qc // AG_GROUP
                    nc.gpsimd.collective_compute(
                        "AllGather", mybir.AluOpType.bypass,
                        replica_groups=[list(range(N_CORES))],
                        ins=[at_loc[g].ap().opt()],
                        outs=[at_full[g].ap().opt()],
                    )

        # ---------------- Phase 3: out-projection ----------------
        with tc.tile_pool(name="af", bufs=2) as afp, \
             tc.tile_pool(name="op_ps", bufs=2, space="PSUM") as op_ps, \
             tc.tile_pool(name="oev", bufs=3) as oevp:
            for qc in range(NSC):
                af = afp.tile([128, NKC * SC], AT, tag="af", name="af")
                nc.sync.dma_start(
                    af[:],
                    at_full[qc // AG_GROUP].ap()
                    .rearrange("(kc p) q -> p kc q", p=128)
                    [:, :, (qc % AG_GROUP) * SC:(qc % AG_GROUP + 1) * SC])
                for oft in range(2):
                    ps = op_ps.tile([128, SC], F32, tag="opps", name="opps")
                    for kc in range(NKC):
                        nc.tensor.matmul(
                            ps[:],
                            wo_sb[:, kc * FL + oft * 128:
                                  kc * FL + (oft + 1) * 128],
                            af[:, kc * SC:(kc + 1) * SC],
                            start=(kc == 0), stop=(kc == NKC - 1),
                            perf_mode=DR)
                    oev = oevp.tile([128, SC], F32, tag="oev", name="oev")
                    nc.scalar.copy(oev[:], ps[:])
                    nc.sync.dma_start(
                        outT.ap()[oft * 128:(oft + 1) * 128,
                                  qc * SC:(qc + 1) * SC], oev[:])


def _get_program():
    if "nc" not in _PROG_CACHE:
        _PROG_CACHE["nc"] = _build_program()
    return _PROG_CACHE["nc"]


def _quad_perm():
    perm = []
    for stv in range(2):
        base = stv * 64
        for quad in range(2):
            for j in range(16):
                perm.append(base + quad * 16 + j)
            for j in range(16):
                perm.append(base + 32 + quad * 16 + j)
    return np.array(perm, dtype=np.int64)


def _host_inputs(x, x_pos, Wq, Wk, Wv, Wo, lq1, lk1, lq2, lk2):
    import ml_dtypes
    BF = ml_dtypes.bfloat16
    ATNP = ml_dtypes.float8_e4m3 if FP8_AT else BF

    x = np.asarray(x, dtype=np.float32)
    xT = np.ascontiguousarray(x.reshape(S, HID).T)

    pos = np.asarray(x_pos, dtype=np.float32).reshape(S)
    inv_freq = (1.0 / (10000.0 ** (np.arange(0, QD, 2, dtype=np.float32)
                                   / QD))).astype(np.float32)
    freqs = pos[:, None] * inv_freq[None, :]
    cos32 = np.cos(freqs).astype(np.float32).T
    sin32 = np.sin(freqs).astype(np.float32).T
    cosT = np.empty((128, S), np.float32)
    sinT = np.empty((128, S), np.float32)
    for r in range(128):
        q, j = divmod(r, 32)
        p = (q % 2) * 16 + (j % 16)
        cosT[r] = cos32[p]
        sinT[r] = sin32[p] if j < 16 else -sin32[p]

    lq1 = np.asarray(lq1, np.float32); lk1 = np.asarray(lk1, np.float32)
    lq2 = np.asarray(lq2, np.float32); lk2 = np.asarray(lk2, np.float32)
    lam = (np.exp(np.sum(lq1 * lk1, dtype=np.float32), dtype=np.float32)
           - np.exp(np.sum(lq2 * lk2, dtype=np.float32), dtype=np.float32)
           + np.float32(LAMBDA_INIT))
    lam = np.array([[lam]], dtype=np.float32)

    cst = np.zeros((128, 4), dtype=np.float32)
    cst[:, 0] = 1.0
    cst[0:64, 1] = 1.0
    cst[64:128, 2] = 1.0
    onesb = np.ones((128, 1), dtype=BF)
    gsel = np.zeros((2, 128), dtype=np.float32)
    gsel[0, 0:64] = 1.0
    gsel[1, 64:128] = 1.0
    ones1 = np.ones((1, 128), dtype=np.float32)

    kk = np.arange(KT, dtype=np.int64)[:, None]
    qq = np.arange(SC, dtype=np.int64)[None, :]
    m01 = np.concatenate(
        [(qq - kk >= off * KT).astype(np.float32)
         for off in range(4)], axis=1).astype(BF)

    Wq = np.asarray(Wq, np.float32); Wk = np.asarray(Wk, np.float32)
    Wv = np.asarray(Wv, np.float32); Wo = np.asarray(Wo, np.float32)

    qp = _quad_perm()
    perm256 = np.concatenate([qp, 128 + qp])

    in_maps = []
    for i in range(N_CORES):
        sl = slice(i * FL, (i + 1) * FL)
        in_maps.append({
            "xT": xT,
            "WqT": np.ascontiguousarray(Wq[sl, :][perm256, :].T),
            "WkT": np.ascontiguousarray(Wk[sl, :][perm256, :].T),
            "WvT": np.ascontiguousarray(Wv[sl, :].T),
            "WoT": np.ascontiguousarray(Wo[sl, :].T).astype(ATNP),
            "cosT": cosT, "sinT": sinT, "m01": m01, "cst": cst,
            "onesb": onesb, "gsel": gsel, "ones1": ones1, "lam": lam,
        })
    return in_maps


def kernel(x, x_pos, Wq, Wk, Wv, Wo, lq1, lk1, lq2, lk2):
    from concourse.bass_utils import run_bass_kernel_spmd

    nc = _get_program()
    in_maps = _host_inputs(x, x_pos, Wq, Wk, Wv, Wo, lq1, lk1, lq2, lk2)
    res = run_bass_kernel_spmd(nc, in_maps, list(range(N_CORES)))
    outT_full = np.concatenate(
        [res.results[c]["outT"] for c in range(N_CORES)], axis=0)
    return np.ascontiguousarray(outT_full.T).reshape(1, S, HID)
